# revision 1
# baseline (speedup 1.0000x reference)
"""Trainium2 Bass kernel for CPUQuantizedLinear (int4 column-parallel linear).

Math: out = x @ w.T + bias, w = (unpack_int4(weight_int4) - zp[:,None]) * scale[:,None]

Key identity used here:
    out[t, o] = scale[o] * (x[t, :] @ (q[o, :] - zp[o])) + bias[o]
(q - zp) is an integer in [-15, 15], exactly representable in bfloat16, so the
device does a pure bf16 matmul (fp32 PSUM accumulation) against integer-valued
bf16 weights, and scale/bias are folded into a single per-partition
tensor_scalar at PSUM eviction.  Only x's bf16 rounding contributes error.

Sharding (column-parallel per hint): out_features 11008 -> 1376 per core,
padded to 1408 = 11 * 128; x replicated, pre-transposed K-major on host.
"""

import sys

if "/opt/trn_rl_repo" not in sys.path:
    sys.path.insert(0, "/opt/trn_rl_repo")

import numpy as np
import ml_dtypes

B, S, IN, OUT = 4, 2048, 4096, 11008
NCORES = 8
P = 128
OUT_PER = OUT // NCORES          # 1376 out-features per core
M_TILES = -(-OUT_PER // P)       # 11 tiles of 128 (padded)
OUT_PAD = M_TILES * P            # 1408
KO = IN // P                     # 32 contraction subtiles
NT = B * S                       # 8192 tokens
N_TILE = 512
N_TILES = NT // N_TILE           # 16


def build(iters: int = 1, ko: int = KO, m_tiles: int = M_TILES, n_tiles: int = N_TILES,
          n_tile: int = N_TILE):
    """Build the per-core Bass module.  iters>1 wraps the compute in a HW loop
    (used by test.py for wall-clock-diff timing; the graded kernel uses 1)."""
    import concourse.bacc as bacc
    import concourse.tile as tile
    import concourse.mybir as mybir

    out_pad = m_tiles * P
    nt = n_tiles * n_tile

    nc = bacc.Bacc("TRN2", target_bir_lowering=False, debug=False)
    xt = nc.dram_tensor("xt", [P, ko, nt], mybir.dt.bfloat16, kind="ExternalInput")
    wq = nc.dram_tensor("wq", [P, ko, out_pad], mybir.dt.bfloat16, kind="ExternalInput")
    sc = nc.dram_tensor("sc", [P, m_tiles], mybir.dt.float32, kind="ExternalInput")
    bs = nc.dram_tensor("bs", [P, m_tiles], mybir.dt.float32, kind="ExternalInput")
    out = nc.dram_tensor("out", [P, m_tiles, nt], mybir.dt.float32, kind="ExternalOutput")

    with tile.TileContext(nc) as tc:
        with (
            tc.tile_pool(name="wpool", bufs=1) as wpool,
            tc.tile_pool(name="xpool", bufs=2) as xpool,
            tc.tile_pool(name="opool", bufs=3) as opool,
            tc.tile_pool(name="cpool", bufs=1) as cpool,
            tc.tile_pool(name="psum", bufs=4, space="PSUM") as psum_pool,
        ):
            sc_sb = cpool.tile([P, m_tiles], mybir.dt.float32)
            bs_sb = cpool.tile([P, m_tiles], mybir.dt.float32)
            nc.sync.dma_start(sc_sb[:], sc[:])
            nc.sync.dma_start(bs_sb[:], bs[:])
            w_sb = wpool.tile([P, ko, out_pad], mybir.dt.bfloat16)
            for k in range(ko):
                nc.sync.dma_start(w_sb[:, k], wq[:, k])

            def body():
                for n in range(n_tiles):
                    x_sb = xpool.tile([P, ko, n_tile], mybir.dt.bfloat16)
                    for k in range(ko):
                        nc.sync.dma_start(
                            x_sb[:, k], xt[:, k, n * n_tile:(n + 1) * n_tile]
                        )
                    for m in range(m_tiles):
                        ps = psum_pool.tile([P, n_tile], mybir.dt.float32)
                        for k in range(ko):
                            nc.tensor.matmul(
                                ps[:],
                                w_sb[:, k, m * P:(m + 1) * P],
                                x_sb[:, k],
                                start=(k == 0),
                                stop=(k == ko - 1),
                            )
                        o_sb = opool.tile([P, n_tile], mybir.dt.float32)
                        nc.vector.tensor_scalar(
                            o_sb[:], ps[:],
                            sc_sb[:, m:m + 1], bs_sb[:, m:m + 1],
                            mybir.AluOpType.mult, mybir.AluOpType.add,
                        )
                        nc.sync.dma_start(
                            out[:, m, n * n_tile:(n + 1) * n_tile], o_sb[:]
                        )

            if iters == 1:
                body()
            else:
                with tc.For_i(0, iters, 1):
                    body()

    nc.compile()
    return nc


def prep_inputs(x, weight_int4, weight_scales, weight_zero_points, bias):
    """Host-side shard + layout.  Returns in_maps (one dict per core)."""
    x = np.asarray(x, dtype=np.float32)
    weight_int4 = np.asarray(weight_int4)
    weight_scales = np.asarray(weight_scales, dtype=np.float32)
    weight_zero_points = np.asarray(weight_zero_points)
    bias = np.asarray(bias, dtype=np.float32)

    # unpack nibbles: low -> even cols, high -> odd cols; subtract zp (exact in bf16)
    low = (weight_int4 & 15).astype(np.int32)
    high = ((weight_int4 >> 4) & 15).astype(np.int32)
    q = np.stack([low, high], axis=-1).reshape(OUT, IN)
    wq_all = (q - weight_zero_points[:, None].astype(np.int32)).astype(
        ml_dtypes.bfloat16
    )  # [OUT, IN], integer-valued, exact

    # x: [B,S,IN] -> bf16 -> [P, KO, NT] with k = ko*128 + p
    xb = x.reshape(NT, IN).astype(ml_dtypes.bfloat16)
    xt = np.ascontiguousarray(xb.reshape(NT, KO, P).transpose(2, 1, 0))

    in_maps = []
    for c in range(NCORES):
        wc = wq_all[c * OUT_PER:(c + 1) * OUT_PER]           # [1376, IN]
        wc_pad = np.zeros((OUT_PAD, IN), dtype=ml_dtypes.bfloat16)
        wc_pad[:OUT_PER] = wc
        wct = np.ascontiguousarray(wc_pad.reshape(OUT_PAD, KO, P).transpose(2, 1, 0))

        sc_pad = np.zeros((OUT_PAD,), dtype=np.float32)
        sc_pad[:OUT_PER] = weight_scales[c * OUT_PER:(c + 1) * OUT_PER]
        bs_pad = np.zeros((OUT_PAD,), dtype=np.float32)
        bs_pad[:OUT_PER] = bias[c * OUT_PER:(c + 1) * OUT_PER]

        in_maps.append({
            "xt": xt,
            "wq": wct,
            "sc": np.ascontiguousarray(sc_pad.reshape(M_TILES, P).T),
            "bs": np.ascontiguousarray(bs_pad.reshape(M_TILES, P).T),
        })
    return in_maps


def gather_output(results):
    """Per-core [P, M_TILES, NT] fp32 -> full [B, S, OUT]."""
    parts = []
    for c in range(NCORES):
        r = results[c]["out"]                                 # [128, 11, 8192]
        parts.append(r.transpose(1, 0, 2).reshape(OUT_PAD, NT)[:OUT_PER])
    full = np.concatenate(parts, axis=0)                      # [11008, 8192]
    return np.ascontiguousarray(full.T).reshape(B, S, OUT).astype(np.float32)


def kernel(x, weight_int4, weight_scales, weight_zero_points, bias):
    from concourse.bass_utils import run_bass_kernel_spmd

    in_maps = prep_inputs(x, weight_int4, weight_scales, weight_zero_points, bias)
    nc = build(iters=1)
    res = run_bass_kernel_spmd(nc, in_maps, core_ids=list(range(NCORES)))
    return gather_output(res.results)


if __name__ == "__main__":
    # quick self-run against random data (no reference)
    rng = np.random.default_rng(0)
    x = rng.standard_normal((B, S, IN), dtype=np.float32)
    w4 = rng.integers(0, 256, (OUT, IN // 2), dtype=np.int32)
    ws = (rng.random(OUT, dtype=np.float32) * 0.01)
    zp = rng.integers(0, 16, (OUT,), dtype=np.int32)
    b = rng.standard_normal(OUT, dtype=np.float32) * 0.1
    out = kernel(x, w4, ws, zp, b)
    print(out.shape, out.dtype)


# revision 2
# speedup vs baseline: 12.4709x; 12.4709x over previous
"""Trainium2 Bass kernel for CPUQuantizedLinear (int4 column-parallel linear).

Math: out = x @ w.T + bias, w = (unpack_int4(weight_int4) - zp[:,None]) * scale[:,None]

Key identity used here:
    out[t, o] = scale[o] * (x[t, :] @ (q[o, :] - zp[o])) + bias[o]
(q - zp) is an integer in [-15, 15], exactly representable in float16, so the
device does a pure fp16 matmul (fp32 PSUM accumulation) against integer-valued
bf16 weights, and scale/bias are folded into a single per-partition
tensor_scalar at PSUM eviction.  Only x's bf16 rounding contributes error.

Sharding (column-parallel per hint): out_features 11008 -> 1376 per core,
padded to 1408 = 11 * 128; x replicated, pre-transposed K-major on host.
"""

import sys

if "/opt/trn_rl_repo" not in sys.path:
    sys.path.insert(0, "/opt/trn_rl_repo")

import numpy as np
import ml_dtypes

B, S, IN, OUT = 4, 2048, 4096, 11008
NCORES = 8
P = 128
OUT_PER = OUT // NCORES          # 1376 out-features per core
M_TILES = -(-OUT_PER // P)       # 11 tiles of 128 (padded)
OUT_PAD = M_TILES * P            # 1408
KO = IN // P                     # 32 contraction subtiles
NT = B * S                       # 8192 tokens
N_TILE = 512
N_TILES = NT // N_TILE           # 16


def build(iters: int = 1, ko: int = KO, m_tiles: int = M_TILES, n_tiles: int = N_TILES,
          n_tile: int = N_TILE):
    """Build the per-core Bass module.  iters>1 wraps the compute in a HW loop
    (used by test.py for wall-clock-diff timing; the graded kernel uses 1)."""
    import concourse.bacc as bacc
    import concourse.tile as tile
    import concourse.mybir as mybir

    out_pad = m_tiles * P
    nt = n_tiles * n_tile

    nc = bacc.Bacc("TRN2", target_bir_lowering=False, debug=False)
    xt = nc.dram_tensor("xt", [P, ko, nt], mybir.dt.float16, kind="ExternalInput")
    wq = nc.dram_tensor("wq", [P, ko, out_pad], mybir.dt.float16, kind="ExternalInput")
    sc = nc.dram_tensor("sc", [P, m_tiles], mybir.dt.float32, kind="ExternalInput")
    bs = nc.dram_tensor("bs", [P, m_tiles], mybir.dt.float32, kind="ExternalInput")
    out = nc.dram_tensor("out", [P, m_tiles, nt], mybir.dt.float32, kind="ExternalOutput")

    with tile.TileContext(nc) as tc:
        with (
            tc.tile_pool(name="wpool", bufs=1) as wpool,
            tc.tile_pool(name="xpool", bufs=2) as xpool,
            tc.tile_pool(name="opool", bufs=3) as opool,
            tc.tile_pool(name="cpool", bufs=1) as cpool,
            tc.tile_pool(name="psum", bufs=4, space="PSUM") as psum_pool,
        ):
            sc_sb = cpool.tile([P, m_tiles], mybir.dt.float32)
            bs_sb = cpool.tile([P, m_tiles], mybir.dt.float32)
            nc.sync.dma_start(sc_sb[:], sc[:])
            nc.sync.dma_start(bs_sb[:], bs[:])
            w_sb = wpool.tile([P, ko, out_pad], mybir.dt.float16)
            for k in range(ko):
                nc.sync.dma_start(w_sb[:, k], wq[:, k])

            def body():
                for n in range(n_tiles):
                    x_sb = xpool.tile([P, ko, n_tile], mybir.dt.float16)
                    for k in range(ko):
                        nc.sync.dma_start(
                            x_sb[:, k], xt[:, k, n * n_tile:(n + 1) * n_tile]
                        )
                    for m in range(m_tiles):
                        ps = psum_pool.tile([P, n_tile], mybir.dt.float32)
                        for k in range(ko):
                            nc.tensor.matmul(
                                ps[:],
                                w_sb[:, k, m * P:(m + 1) * P],
                                x_sb[:, k],
                                start=(k == 0),
                                stop=(k == ko - 1),
                            )
                        o_sb = opool.tile([P, n_tile], mybir.dt.float32)
                        nc.vector.tensor_scalar(
                            o_sb[:], ps[:],
                            sc_sb[:, m:m + 1], bs_sb[:, m:m + 1],
                            mybir.AluOpType.mult, mybir.AluOpType.add,
                        )
                        nc.sync.dma_start(
                            out[:, m, n * n_tile:(n + 1) * n_tile], o_sb[:]
                        )

            if iters == 1:
                body()
            else:
                with tc.For_i(0, iters, 1):
                    body()

    nc.compile()
    return nc


def prep_inputs(x, weight_int4, weight_scales, weight_zero_points, bias):
    """Host-side shard + layout.  Returns in_maps (one dict per core)."""
    x = np.asarray(x, dtype=np.float32)
    weight_int4 = np.asarray(weight_int4)
    weight_scales = np.asarray(weight_scales, dtype=np.float32)
    weight_zero_points = np.asarray(weight_zero_points)
    bias = np.asarray(bias, dtype=np.float32)

    # unpack nibbles: low -> even cols, high -> odd cols; subtract zp (exact in bf16)
    low = (weight_int4 & 15).astype(np.int32)
    high = ((weight_int4 >> 4) & 15).astype(np.int32)
    q = np.stack([low, high], axis=-1).reshape(OUT, IN)
    wq_all = (q - weight_zero_points[:, None].astype(np.int32)).astype(
        np.float16
    )  # [OUT, IN], integer-valued, exact

    # x: [B,S,IN] -> bf16 -> [P, KO, NT] with k = ko*128 + p
    xb = x.reshape(NT, IN).astype(np.float16)
    xt = np.ascontiguousarray(xb.reshape(NT, KO, P).transpose(2, 1, 0))

    in_maps = []
    for c in range(NCORES):
        wc = wq_all[c * OUT_PER:(c + 1) * OUT_PER]           # [1376, IN]
        wc_pad = np.zeros((OUT_PAD, IN), dtype=np.float16)
        wc_pad[:OUT_PER] = wc
        wct = np.ascontiguousarray(wc_pad.reshape(OUT_PAD, KO, P).transpose(2, 1, 0))

        sc_pad = np.zeros((OUT_PAD,), dtype=np.float32)
        sc_pad[:OUT_PER] = weight_scales[c * OUT_PER:(c + 1) * OUT_PER]
        bs_pad = np.zeros((OUT_PAD,), dtype=np.float32)
        bs_pad[:OUT_PER] = bias[c * OUT_PER:(c + 1) * OUT_PER]

        in_maps.append({
            "xt": xt,
            "wq": wct,
            "sc": np.ascontiguousarray(sc_pad.reshape(M_TILES, P).T),
            "bs": np.ascontiguousarray(bs_pad.reshape(M_TILES, P).T),
        })
    return in_maps


def gather_output(results):
    """Per-core [P, M_TILES, NT] fp32 -> full [B, S, OUT]."""
    parts = []
    for c in range(NCORES):
        r = results[c]["out"]                                 # [128, 11, 8192]
        parts.append(r.transpose(1, 0, 2).reshape(OUT_PAD, NT)[:OUT_PER])
    full = np.concatenate(parts, axis=0)                      # [11008, 8192]
    return np.ascontiguousarray(full.T).reshape(B, S, OUT).astype(np.float32)


def kernel(x, weight_int4, weight_scales, weight_zero_points, bias):
    from concourse.bass_utils import run_bass_kernel_spmd

    in_maps = prep_inputs(x, weight_int4, weight_scales, weight_zero_points, bias)
    nc = build(iters=1)
    res = run_bass_kernel_spmd(nc, in_maps, core_ids=list(range(NCORES)))
    return gather_output(res.results)


if __name__ == "__main__":
    # quick self-run against random data (no reference)
    rng = np.random.default_rng(0)
    x = rng.standard_normal((B, S, IN), dtype=np.float32)
    w4 = rng.integers(0, 256, (OUT, IN // 2), dtype=np.int32)
    ws = (rng.random(OUT, dtype=np.float32) * 0.01)
    zp = rng.integers(0, 16, (OUT,), dtype=np.int32)
    b = rng.standard_normal(OUT, dtype=np.float32) * 0.1
    out = kernel(x, w4, ws, zp, b)
    print(out.shape, out.dtype)


# revision 6
# speedup vs baseline: 15.3750x; 1.2329x over previous
"""Trainium2 Bass kernel for CPUQuantizedLinear (int4 column-parallel linear).

Math: out = x @ w.T + bias, w = (unpack_int4(weight_int4) - zp[:,None]) * scale[:,None]

Key identity used here:
    out[t, o] = scale[o] * (x[t, :] @ (q[o, :] - zp[o])) + bias[o]
(q - zp) is an integer in [-15, 15], exactly representable in float16, so the
device does a pure fp16 matmul (fp32 PSUM accumulation) against integer-valued
bf16 weights, and scale/bias are folded into a single per-partition
tensor_scalar at PSUM eviction.  Only x's bf16 rounding contributes error.

Sharding (column-parallel per hint): out_features 11008 -> 1376 per core,
padded to 1408 = 11 * 128; x replicated, pre-transposed K-major on host.
"""

import sys

if "/opt/trn_rl_repo" not in sys.path:
    sys.path.insert(0, "/opt/trn_rl_repo")

import numpy as np
import ml_dtypes

B, S, IN, OUT = 4, 2048, 4096, 11008
NCORES = 8
P = 128
OUT_PER = OUT // NCORES          # 1376 out-features per core
M_TILES = -(-OUT_PER // P)       # 11 tiles of 128 (padded)
OUT_PAD = M_TILES * P            # 1408
KO = IN // P                     # 32 contraction subtiles
NT = B * S                       # 8192 tokens
N_TILE = 512
N_TILES = NT // N_TILE           # 16


def build(iters: int = 1, ko: int = KO, m_tiles: int = M_TILES, n_tiles: int = N_TILES,
          n_tile: int = N_TILE):
    """Build the per-core Bass module.  iters>1 wraps the compute in a HW loop
    (used by test.py for wall-clock-diff timing; the graded kernel uses 1)."""
    import concourse.bacc as bacc
    import concourse.tile as tile
    import concourse.mybir as mybir

    out_pad = m_tiles * P
    nt = n_tiles * n_tile

    nc = bacc.Bacc("TRN2", target_bir_lowering=False, debug=False)
    # x laid out so one n-tile = one contiguous 32KB-per-partition (4MB) DMA
    xt = nc.dram_tensor("xt", [P, n_tiles, ko, n_tile], mybir.dt.float16,
                        kind="ExternalInput")
    wq = nc.dram_tensor("wq", [P, ko, out_pad], mybir.dt.float16, kind="ExternalInput")
    sc = nc.dram_tensor("sc", [P, m_tiles], mybir.dt.float32, kind="ExternalInput")
    bs = nc.dram_tensor("bs", [P, m_tiles], mybir.dt.float32, kind="ExternalInput")
    out = nc.dram_tensor("out", [P, m_tiles, nt], mybir.dt.float32, kind="ExternalOutput")

    with tile.TileContext(nc) as tc:
        with (
            tc.tile_pool(name="wpool", bufs=1) as wpool,
            tc.tile_pool(name="xpool", bufs=2) as xpool,
            tc.tile_pool(name="opool", bufs=3) as opool,
            tc.tile_pool(name="cpool", bufs=1) as cpool,
            tc.tile_pool(name="psum", bufs=4, space="PSUM") as psum_pool,
        ):
            sc_sb = cpool.tile([P, m_tiles], mybir.dt.float32)
            bs_sb = cpool.tile([P, m_tiles], mybir.dt.float32)
            nc.sync.dma_start(sc_sb[:], sc[:])
            nc.sync.dma_start(bs_sb[:], bs[:])
            w_sb = wpool.tile([P, ko, out_pad], mybir.dt.float16)
            nc.sync.dma_start(w_sb[:], wq[:])

            def body():
                for n in range(n_tiles):
                    x_sb = xpool.tile([P, ko, n_tile], mybir.dt.float16)
                    nc.sync.dma_start(x_sb[:], xt[:, n])
                    for m in range(m_tiles):
                        ps = psum_pool.tile([P, n_tile], mybir.dt.float32)
                        for k in range(ko):
                            nc.tensor.matmul(
                                ps[:],
                                w_sb[:, k, m * P:(m + 1) * P],
                                x_sb[:, k],
                                start=(k == 0),
                                stop=(k == ko - 1),
                            )
                        o_sb = opool.tile([P, n_tile], mybir.dt.float32)
                        nc.vector.tensor_scalar(
                            o_sb[:], ps[:],
                            sc_sb[:, m:m + 1], bs_sb[:, m:m + 1],
                            mybir.AluOpType.mult, mybir.AluOpType.add,
                        )
                        # ACT HWDGE ring: keep stores off the SP ring that
                        # carries the x loads
                        nc.scalar.dma_start(
                            out[:, m, n * n_tile:(n + 1) * n_tile], o_sb[:]
                        )

            if iters == 1:
                body()
            else:
                with tc.For_i(0, iters, 1):
                    body()

    nc.compile()
    return nc


def prep_inputs(x, weight_int4, weight_scales, weight_zero_points, bias):
    """Host-side shard + layout.  Returns in_maps (one dict per core)."""
    x = np.asarray(x, dtype=np.float32)
    weight_int4 = np.asarray(weight_int4)
    weight_scales = np.asarray(weight_scales, dtype=np.float32)
    weight_zero_points = np.asarray(weight_zero_points)
    bias = np.asarray(bias, dtype=np.float32)

    # unpack nibbles: low -> even cols, high -> odd cols; subtract zp (exact in bf16)
    low = (weight_int4 & 15).astype(np.int32)
    high = ((weight_int4 >> 4) & 15).astype(np.int32)
    q = np.stack([low, high], axis=-1).reshape(OUT, IN)
    wq_all = (q - weight_zero_points[:, None].astype(np.int32)).astype(
        np.float16
    )  # [OUT, IN], integer-valued, exact

    # x: [B,S,IN] -> fp16 -> [P, N_TILES, KO, N_TILE]:
    # xt[p, n, k, j] = x[n*N_TILE + j, k*128 + p]
    xb = x.reshape(NT, IN).astype(np.float16)
    xt = np.ascontiguousarray(
        xb.reshape(N_TILES, N_TILE, KO, P).transpose(3, 0, 2, 1))

    in_maps = []
    for c in range(NCORES):
        wc = wq_all[c * OUT_PER:(c + 1) * OUT_PER]           # [1376, IN]
        wc_pad = np.zeros((OUT_PAD, IN), dtype=np.float16)
        wc_pad[:OUT_PER] = wc
        wct = np.ascontiguousarray(wc_pad.reshape(OUT_PAD, KO, P).transpose(2, 1, 0))

        sc_pad = np.zeros((OUT_PAD,), dtype=np.float32)
        sc_pad[:OUT_PER] = weight_scales[c * OUT_PER:(c + 1) * OUT_PER]
        bs_pad = np.zeros((OUT_PAD,), dtype=np.float32)
        bs_pad[:OUT_PER] = bias[c * OUT_PER:(c + 1) * OUT_PER]

        in_maps.append({
            "xt": xt,
            "wq": wct,
            "sc": np.ascontiguousarray(sc_pad.reshape(M_TILES, P).T),
            "bs": np.ascontiguousarray(bs_pad.reshape(M_TILES, P).T),
        })
    return in_maps


def gather_output(results):
    """Per-core [P, M_TILES, NT] fp32 -> full [B, S, OUT]."""
    parts = []
    for c in range(NCORES):
        r = results[c]["out"]                                 # [128, 11, 8192]
        parts.append(r.transpose(1, 0, 2).reshape(OUT_PAD, NT)[:OUT_PER])
    full = np.concatenate(parts, axis=0)                      # [11008, 8192]
    return np.ascontiguousarray(full.T).reshape(B, S, OUT).astype(np.float32)


def kernel(x, weight_int4, weight_scales, weight_zero_points, bias):
    from concourse.bass_utils import run_bass_kernel_spmd

    in_maps = prep_inputs(x, weight_int4, weight_scales, weight_zero_points, bias)
    nc = build(iters=1)
    res = run_bass_kernel_spmd(nc, in_maps, core_ids=list(range(NCORES)))
    return gather_output(res.results)


if __name__ == "__main__":
    # quick self-run against random data (no reference)
    rng = np.random.default_rng(0)
    x = rng.standard_normal((B, S, IN), dtype=np.float32)
    w4 = rng.integers(0, 256, (OUT, IN // 2), dtype=np.int32)
    ws = (rng.random(OUT, dtype=np.float32) * 0.01)
    zp = rng.integers(0, 16, (OUT,), dtype=np.int32)
    b = rng.standard_normal(OUT, dtype=np.float32) * 0.1
    out = kernel(x, w4, ws, zp, b)
    print(out.shape, out.dtype)
